# revision 1
# baseline (speedup 1.0000x reference)
"""Trainium2 Bass kernel for nn_MEX_41386304864396 (dense transformer block).

Sharding: data-parallel over batch B=8 across 8 NeuronCores (one batch element
per core); weights replicated.  Host pre-transposes activations to [D, S] and
pre-folds constants so the device never transposes:
  * residual fold      W' = I + W                  (embed blocks LN(x + xW + b))
  * LN mean fold       extra matmul column: m = x @ (W'.sum(1)/D) + mean(b)
  * LN affine fold     gamma/beta folded into the downstream q/k/v weights
  * v-bias fold        bd' = bd + (bv_g + bv_p) @ Wd   (softmax rows sum to 1)
  * bridged attention  glb_ctx + plb_ctx = softmax_g @ (vg + softmax_p @ vp)
    (associativity removes the S x S x S 'enhanced' matmul)
Scores are computed transposed [key, query]; softmax uses exp without max
subtraction (scores bounded ~|3.4|) and normalization is deferred:
Z_p is a folded ones-column of the U_p matmul, Z_g a folded ones-row of U_g,
and 1/Z_g is partition-broadcast with a K=1 matmul against a ones row.
All matmuls run in float32r (fp32 storage, full-rate PE).  SBUF pools are
two-sided LIFO stacks scoped to phases to stay inside the 192KB partition.
"""
import os
import sys

sys.path.insert(0, '/opt/trn_rl_repo')

import numpy as np

import concourse.bass as bass  # noqa: F401
import concourse.tile as tile
from concourse import bacc, mybir
from concourse import bass2jax

F32 = mybir.dt.float32
F32R = mybir.dt.float32r
AF = mybir.ActivationFunctionType
ALU = mybir.AluOpType

S, B, D, H, DH, FF = 512, 8, 1024, 16, 64, 4096
NK = D // 128
NT = S // 128
NFF = FF // 128
EPS = 1e-5
SCALE = 1.0 / 8.0

DT = F32R


def _declare(nc, timing=False):
    dram = {}
    kind = "Internal" if timing else "ExternalInput"

    def din(name, shape, dt=DT):
        dram[name] = nc.dram_tensor(name, list(shape), dt, kind=kind)

    for n in ("xg", "xl", "xt"):
        din(n, (D, S))
    for n in ("we_l1", "we_l2", "we_g", "w_qg", "w_kg", "w_qp", "w_kp",
              "w_vg", "w_vp", "w_d", "w_ml"):
        din(n, (D, D))
    din("w_fc", (NFF, NK, 128, 128))
    din("w_proj", (NK, NFF, 128, 128))
    for e in ("l1", "l2", "g"):
        din(f"wmean_{e}", (128, NK))
        din(f"bcol_{e}", (128, NK), F32)
    for n in ("bcol_qg", "bcol_kg", "bcol_qp", "bcol_kp", "bcol_d", "bcol_proj"):
        din(n, (128, NK), F32)
    din("bcol_fc", (128, NFF), F32)
    din("bml_bc", (128, D), F32)
    dram["y"] = nc.dram_tensor("y", [S, D], F32, kind="ExternalOutput")
    return dram


def _body(nc, tc, dram, mean_b):
    def pool(name, bufs, side="left", space="SBUF"):
        return tc.alloc_tile_pool(name=name, bufs=bufs, side=side, space=space)

    # ---- global pools ----
    consts = pool("consts", 1)
    rows = pool("rows", 1)
    tmp = pool("tmp", 4)
    sq = pool("sqp", 3)
    small = pool("small", 3)
    outp = pool("outp", 2)

    psA = pool("psA", 4, space="PSUM")
    psRow = pool("psRow", 2, space="PSUM")
    psUp = pool("psUp", 1, space="PSUM")
    psUg = pool("psUg", 1, space="PSUM")

    def mmtile():
        return psA.tile([128, 512], F32, tag="mm", name="mm")

    # ---- constants ----
    ones_f = consts.tile([128, 32], F32, tag="ones_f", name="ones_f")
    nc.vector.memset(ones_f[:], 1.0)
    ones_dt = consts.tile([128, 32], DT, tag="ones_dt", name="ones_dt")
    nc.vector.tensor_copy(ones_dt[:], ones_f[:])
    onesr_f = consts.tile([1, 128], F32, tag="onesr_f", name="onesr_f")
    nc.vector.memset(onesr_f[:], 1.0)
    ones_row = consts.tile([1, 128], DT, tag="ones_row", name="ones_row")
    nc.vector.tensor_copy(ones_row[:], onesr_f[:])
    eps_t = consts.tile([1, 1], F32, tag="eps_t", name="eps_t")
    nc.vector.memset(eps_t[:], EPS)

    def cload(name, shape, dt=F32):
        t = consts.tile(list(shape), dt, tag=name)
        nc.sync.dma_start(out=t[:], in_=dram[name].ap())
        return t

    wmean = {e: cload(f"wmean_{e}", (128, NK), DT) for e in ("l1", "l2", "g")}
    bcol_e = {e: cload(f"bcol_{e}", (128, NK)) for e in ("l1", "l2", "g")}
    bcols = {n: cload(f"bcol_{n}", (128, NK))
             for n in ("qg", "kg", "qp", "kp", "d", "proj")}
    bcol_fc = cload("bcol_fc", (128, NFF))
    bml_bc = cload("bml_bc", (128, D))

    def xload(dname, p, tagp):
        ts = []
        for k in range(NK):
            t = p.tile([128, S], DT, tag=f"{tagp}{k}", name=f"{tagp}{k}")
            nc.sync.dma_start(out=t[:], in_=dram[dname].ap()[k * 128:(k + 1) * 128, :])
            ts.append(t)
        return ts

    def load_wchunks(wp, name):
        ws = []
        for k in range(NK):
            t = wp.tile([128, D], DT, tag=f"w{k}", name=f"w{k}")
            nc.sync.dma_start(out=t[:], in_=dram[name].ap()[k * 128:(k + 1) * 128, :])
            ws.append(t)
        return ws

    # ---- embeds (plain LN; gamma/beta folded downstream on host) ----
    e_state = {}

    def embedA(e, x, wt, pyln):
        mp = psRow.tile([1, 512], F32, tag="row", name="mp")
        for k in range(NK):
            nc.tensor.matmul(mp[:], wmean[e][:, k:k + 1], x[k],
                             start=(k == 0), stop=(k == NK - 1))
        ss = psRow.tile([1, 512], F32, tag="row", name="ss")
        ys = []
        pend = []
        for m in range(NK):
            ps = mmtile()
            for j in range(NK):
                k = (m + 1 + j) % NK
                nc.tensor.matmul(ps[:], wt[k][:, m * 128:(m + 1) * 128], x[k],
                                 start=(j == 0), stop=(j == NK - 1))
            y = pyln.tile([128, 512], F32, tag=f"y{m}", name=f"y{m}")
            nc.vector.tensor_scalar_add(y[:], ps[:], bcol_e[e][:, m:m + 1])
            s = sq.tile([128, 512], DT, tag="sq", name="sq")
            nc.scalar.activation(s[:], ps[:], AF.Square,
                                 bias=bcol_e[e][:, m:m + 1], scale=1.0)
            pend.append(s)
            if m > 0:
                s0 = pend.pop(0)
                nc.tensor.matmul(ss[:], ones_dt[:, 0:1], s0[:],
                                 start=(m == 1), stop=False)
            ys.append(y)
        s0 = pend.pop(0)
        nc.tensor.matmul(ss[:], ones_dt[:, 0:1], s0[:], start=False, stop=True)
        e_state[e] = (mp, ss, ys)

    def embedB(e, epool):
        mp, ss, ys = e_state[e]
        m_sb = rows.tile([1, 512], F32, tag="m_sb", name="m_sb")
        nc.vector.tensor_scalar_add(m_sb[:], mp[:], float(mean_b[e]))
        msq = rows.tile([1, 512], F32, tag="msq", name="msq")
        nc.vector.tensor_mul(msq[:], m_sb[:], m_sb[:])
        var = rows.tile([1, 512], F32, tag="var", name="var")
        nc.vector.scalar_tensor_tensor(out=var[:], in0=ss[:], scalar=1.0 / D,
                                       in1=msq[:], op0=ALU.mult, op1=ALU.subtract)
        std = rows.tile([1, 512], F32, tag="std", name="std")
        nc.scalar.activation(std[:], var[:], AF.Sqrt, bias=eps_t[:], scale=1.0)
        rstd = rows.tile([1, 512], DT, tag="rstd", name="rstd")
        nc.vector.reciprocal(rstd[:], std[:])
        mr = rows.tile([1, 512], DT, tag="mr", name="mr")
        nc.vector.tensor_mul(mr[:], m_sb[:], rstd[:].bitcast(F32))
        et = []
        for m in range(NK):
            aps = mmtile()
            nc.tensor.matmul(aps[:], ones_row[:], rstd[:], start=True, stop=True)
            bps = mmtile()
            nc.tensor.matmul(bps[:], ones_row[:], mr[:], start=True, stop=True)
            t1 = tmp.tile([128, 512], F32, tag="t1", name="t1")
            nc.vector.tensor_mul(t1[:], ys[m][:], aps[:])
            em = epool.tile([128, 512], DT, tag=f"e{m}", name=f"e{m}")
            nc.vector.tensor_sub(em[:], t1[:], bps[:])
            et.append(em)
        e_state[e] = et

    # LEFT: G | w1a, px_gl, pyln   RIGHT: px_t, pe_dup, pe_l2, pe_g2
    px_t = pool("px_t", 1, side="right")
    w1a = pool("w1a", 1)
    px_gl = pool("px_gl", 1)
    pyln = pool("pyln", 1)

    xt = xload("xt", px_t, "xt")
    xg = xload("xg", px_gl, "xg")
    xl = xload("xl", px_gl, "xl")

    embedA("l1", xl, load_wchunks(w1a, "we_l1"), pyln)
    pe_dup = pool("pe_dup", 1, side="right")
    embedB("l1", pe_dup)
    embedA("l2", xl, load_wchunks(w1a, "we_l2"), pyln)
    pe_l2 = pool("pe_l2", 1, side="right")
    embedB("l2", pe_l2)
    embedA("g", xg, load_wchunks(w1a, "we_g"), pyln)
    pe_g2 = pool("pe_g2", 1, side="right")
    embedB("g", pe_g2)
    dupT, l2T, g2T = e_state["l1"], e_state["l2"], e_state["g"]
    pyln.release()
    px_gl.release()
    w1a.release()

    # ---- projections ----
    def projB(wname, src, bcol, opool, tagp):
        wp = pool(f"pw_{tagp}", 1)
        wt = load_wchunks(wp, wname)
        out = []
        for m in range(NK):
            ps = mmtile()
            for j in range(NK):
                k = (m + 1 + j) % NK
                nc.tensor.matmul(ps[:], wt[k][:, m * 128:(m + 1) * 128], src[k],
                                 start=(j == 0), stop=(j == NK - 1))
            o = opool.tile([128, 512], DT, tag=f"{tagp}{m}", name=f"{tagp}{m}")
            nc.vector.tensor_scalar_add(o[:], ps[:], bcol[:, m:m + 1])
            out.append(o)
        wp.release()
        return out

    def projA(wname, src, opool, tagp, width):
        wp = pool(f"pw_{tagp}", 1)
        wt = load_wchunks(wp, wname)
        out = []
        for rt in range(NT):
            vt = opool.tile([128, H, width], DT, tag=f"{tagp}{rt}", name=f"{tagp}{rt}")
            if width == DH + 2:
                for c in (DH, DH + 1):
                    nc.vector.tensor_copy(
                        vt[:, :, c:c + 1].rearrange("p h one -> p (h one)"),
                        ones_dt[:, 0:H])
            for half in range(2):
                ps = mmtile()
                for j in range(NK):
                    k = (2 * rt + half + 1 + j) % NK
                    nc.tensor.matmul(
                        ps[:], src[k][:, rt * 128:(rt + 1) * 128],
                        wt[k][:, half * 512:(half + 1) * 512],
                        start=(j == 0), stop=(j == NK - 1))
                nc.vector.tensor_copy(
                    vt[:, half * 8:(half + 1) * 8, 0:DH],
                    ps[:].rearrange("p (h d) -> p h d", h=8))
            out.append(vt)
        wp.release()
        return out

    pqg = pool("pqg", 1)
    qgT = projB("w_qg", g2T, bcols["qg"], pqg, "qg")
    pe_g2.release()
    pkg = pool("pkg", 1)
    kgT = projB("w_kg", l2T, bcols["kg"], pkg, "kg")
    pv = pool("pv", 1)
    vg = projA("w_vg", l2T, pv, "vg", DH)
    pe_l2.release()
    pqp = pool("pqp", 1)
    qpT = projB("w_qp", dupT, bcols["qp"], pqp, "qp")
    pe_dup.release()
    pkp = pool("pkp", 1)
    kpT = projB("w_kp", xt, bcols["kp"], pkp, "kp")
    vp = projA("w_vp", xt, pv, "vpn", DH + 2)
    px_t.release()

    # ---- attention ----
    pctx = pool("pctx", 1, side="right")
    pexp = pool("pexp", 2, side="right")

    def hsl(tiles, h):
        return tiles[h // 2][64 * (h % 2):64 * (h % 2) + 64, :]

    ctxT = [pctx.tile([128, 512], DT, tag=f"ctx{j}", name=f"ctx{j}")
            for j in range(NK)]
    stage1_out = {}

    def attn_stage1(h):
        qg_h, kg_h = hsl(qgT, h), hsl(kgT, h)
        qp_h, kp_h = hsl(qpT, h), hsl(kpT, h)
        ep = []
        for kt in range(NT):
            sp = mmtile()
            nc.tensor.matmul(sp[:], kp_h[:, kt * 128:(kt + 1) * 128], qp_h,
                             start=True, stop=True)
            e = pexp.tile([128, 512], DT, tag=f"ep{kt}", name=f"ep{kt}")
            nc.scalar.activation(e[:], sp[:], AF.Exp, scale=SCALE)
            ep.append(e)
        eg = []
        for kt in range(NT):
            sg = mmtile()
            nc.tensor.matmul(sg[:], kg_h[:, kt * 128:(kt + 1) * 128], qg_h,
                             start=True, stop=True)
            e = pexp.tile([128, 512], DT, tag=f"eg{kt}", name=f"eg{kt}")
            nc.scalar.activation(e[:], sg[:], AF.Exp, scale=SCALE)
            eg.append(e)
        up = psUp.tile([128, NT, DH + 2], F32, tag="up", name="up")
        vph = []
        for kt in range(NT):
            for tt in range(NT):
                nc.tensor.matmul(up[:, kt, :], ep[tt][:, kt * 128:(kt + 1) * 128],
                                 vp[tt][:, h, 0:DH + 2], start=(tt == 0),
                                 stop=(tt == NT - 1))
            rp = small.tile([128, 1], F32, tag="rp", name="rp")
            nc.vector.reciprocal(rp[:], up[:, kt, DH:DH + 1])
            vt = pexp.tile([128, DH + 1], DT, tag=f"vph{kt}", name=f"vph{kt}")
            nc.vector.scalar_tensor_tensor(
                out=vt[:, 0:DH], in0=up[:, kt, 0:DH], scalar=rp[:],
                in1=vg[kt][:, h, :], op0=ALU.mult, op1=ALU.add)
            nc.vector.tensor_copy(vt[:, DH:DH + 1], ones_dt[:, 0:1])
            vph.append(vt)
        stage1_out[h] = (eg, vph)

    def attn_stage2(h):
        eg, vph = stage1_out.pop(h)
        ug = psUg.tile([DH + 1, 512], F32, tag="ug", name="ug")
        for kt in range(NT):
            nc.tensor.matmul(ug[:], vph[kt][:], eg[kt][:],
                             start=(kt == 0), stop=(kt == NT - 1))
        rg = rows.tile([1, 512], DT, tag="rg", name="rg")
        nc.vector.reciprocal(rg[:], ug[DH:DH + 1, :])
        rb = mmtile()
        nc.tensor.matmul(rb[0:64, :], ones_row[0:1, 0:64], rg[:],
                         start=True, stop=True)
        rbs = tmp.tile([64, 512], F32, tag="t1", name="rbs")
        nc.scalar.activation(rbs[:], rb[0:64, :], AF.Copy)
        off = 64 * (h % 2)
        nc.vector.tensor_mul(ctxT[h // 2][off:off + 64, :], ug[0:DH, :], rbs[:])

    attn_stage1(0)
    for h in range(H):
        if h + 1 < H:
            attn_stage1(h + 1)
        attn_stage2(h)
    pexp.release()
    pkp.release()
    pqp.release()
    pv.release()
    pkg.release()
    pqg.release()

    # ---- out1 = ctx @ Wd + bd' ----
    po1 = pool("po1", 1)
    out1T = projB("w_d", ctxT, bcols["d"], po1, "o1")
    pctx.release()

    # ---- MLP ----
    ph1 = pool("ph1", 1)
    pwfc = pool("pwfc", 2)
    h1 = []
    for ff in range(NFF):
        wt = pwfc.tile([128, NK, 128], DT, tag="wfc", name="wfc")
        nc.sync.dma_start(out=wt[:],
                          in_=dram["w_fc"].ap()[ff].rearrange("k p n -> p k n"))
        ps = mmtile()
        for k in range(NK):
            nc.tensor.matmul(ps[:], wt[:, k, :], out1T[k],
                             start=(k == 0), stop=(k == NK - 1))
        g = ph1.tile([128, 512], DT, tag=f"h1_{ff}", name=f"h1_{ff}")
        nc.scalar.activation(g[:], ps[:], AF.Gelu,
                             bias=bcol_fc[:, ff:ff + 1], scale=1.0)
        h1.append(g)
    pwfc.release()

    po2 = pool("po2", 1)
    pwpj = pool("pwpj", 2)
    out2T = []
    for m in range(NK):
        wta = pwpj.tile([128, NFF // 2, 128], DT, tag="wpj", name="wpja")
        wtb = pwpj.tile([128, NFF // 2, 128], DT, tag="wpj", name="wpjb")
        nc.sync.dma_start(
            out=wta[:], in_=dram["w_proj"].ap()[m, 0:NFF // 2].rearrange("k p n -> p k n"))
        nc.sync.dma_start(
            out=wtb[:], in_=dram["w_proj"].ap()[m, NFF // 2:NFF].rearrange("k p n -> p k n"))
        ps = mmtile()
        for k in range(NFF):
            w = wta if k < NFF // 2 else wtb
            nc.tensor.matmul(ps[:], w[:, k % (NFF // 2), :], h1[k],
                             start=(k == 0), stop=(k == NFF - 1))
        o = po2.tile([128, 512], DT, tag=f"o2m{m}", name=f"o2m{m}")
        nc.vector.tensor_scalar_add(o[:], ps[:], bcols["proj"][:, m:m + 1])
        out2T.append(o)
    pwpj.release()

    # ---- y = out2 @ Wml + bml (natural layout) ----
    pw3 = pool("pw3", 1)
    wml = load_wchunks(pw3, "w_ml")
    for rt in range(NT):
        yt = outp.tile([128, D], F32, tag="yout", name="yout")
        for half in range(2):
            ps = mmtile()
            for j in range(NK):
                k = (2 * rt + half + 1 + j) % NK
                nc.tensor.matmul(ps[:], out2T[k][:, rt * 128:(rt + 1) * 128],
                                 wml[k][:, half * 512:(half + 1) * 512],
                                 start=(j == 0), stop=(j == NK - 1))
            nc.vector.tensor_add(yt[:, half * 512:(half + 1) * 512], ps[:],
                                 bml_bc[:, half * 512:(half + 1) * 512])
        nc.sync.dma_start(out=dram["y"].ap()[rt * 128:(rt + 1) * 128, :], in_=yt[:])
    pw3.release()
    po2.release()
    ph1.release()
    po1.release()

    for p in (outp, small, sq, tmp, rows, consts, psUg, psUp, psRow, psA):
        p.release()


def build(repeat=1, mean_b=None, timing=False):
    mean_b = mean_b or {"l1": 0.0, "l2": 0.0, "g": 0.0}
    nc = bacc.Bacc(None, target_bir_lowering=False, debug=False)
    dram = _declare(nc, timing=timing)
    with tile.TileContext(nc) as tc:
        with nc.allow_low_precision(reason="float32r storage is fp32-width"):
            if repeat > 1:
                with tc.For_i(0, repeat, 1):
                    _body(nc, tc, dram, mean_b)
            else:
                _body(nc, tc, dram, mean_b)
    nc.compile()

    class CX:
        pass

    cx = CX()
    cx.nc = nc
    cx.dram = dram
    return cx


# ---------------------------------------------------------------------------
# host side
# ---------------------------------------------------------------------------

def _prep_host(inputs):
    f32 = np.float32
    g = np.asarray(inputs["global_feat"], f32)
    l = np.asarray(inputs["local_feat"], f32)
    t = np.asarray(inputs["text_feat"], f32)
    W = {k: np.asarray(inputs[k], f32) for k in
         ("Wg_emb", "Wl1", "Wl2", "Wq_g", "Wk_g", "Wv_g", "Wq_p", "Wk_p",
          "Wv_p", "Wd", "Wml", "Wfc", "Wproj")}
    bv = {k: np.asarray(inputs[k], f32) for k in
          ("bg_emb", "bl1", "bl2", "bq_g", "bk_g", "bv_g", "bq_p", "bk_p",
           "bv_p", "bd", "bml", "bproj", "bfc",
           "betag_emb", "betal1", "betal2", "gg_emb", "gl1", "gl2")}

    I = np.eye(D, dtype=f32)
    shared = {}
    mean_b = {}
    for e, (wn, bn) in {"l1": ("Wl1", "bl1"), "l2": ("Wl2", "bl2"),
                        "g": ("Wg_emb", "bg_emb")}.items():
        Wp = (I + W[wn]).astype(f32)
        shared[f"we_{e}"] = np.ascontiguousarray(Wp)
        shared[f"wmean_{e}"] = np.ascontiguousarray(
            (Wp.sum(axis=1) / D).astype(f32).reshape(NK, 128).T)
        mean_b[e] = float(bv[bn].mean())
        shared[f"bcol_{e}"] = np.ascontiguousarray(bv[bn].reshape(NK, 128).T)

    # fold LN gamma/beta of the producing embed into each consumer projection
    def foldp(Wname, bname, gamma, beta):
        Wf = (gamma[:, None] * W[Wname]).astype(f32)
        bf = (np.asarray(bv[bname]) + beta @ W[Wname]).astype(f32)
        return Wf, bf

    w_qg, b_qg = foldp("Wq_g", "bq_g", bv["gg_emb"], bv["betag_emb"])
    w_kg, b_kg = foldp("Wk_g", "bk_g", bv["gl2"], bv["betal2"])
    w_vg, b_vg = foldp("Wv_g", "bv_g", bv["gl2"], bv["betal2"])
    w_qp, b_qp = foldp("Wq_p", "bq_p", bv["gl1"], bv["betal1"])
    shared["w_qg"] = np.ascontiguousarray(w_qg)
    shared["w_kg"] = np.ascontiguousarray(w_kg)
    shared["w_qp"] = np.ascontiguousarray(w_qp)
    shared["w_kp"] = np.ascontiguousarray(W["Wk_p"])
    shared["w_vg"] = np.ascontiguousarray(w_vg)
    shared["w_vp"] = np.ascontiguousarray(W["Wv_p"])
    shared["w_d"] = np.ascontiguousarray(W["Wd"])
    shared["w_ml"] = np.ascontiguousarray(W["Wml"])
    for n, b_ in (("bcol_qg", b_qg), ("bcol_kg", b_kg), ("bcol_qp", b_qp),
                  ("bcol_kp", bv["bk_p"]), ("bcol_proj", bv["bproj"])):
        shared[n] = np.ascontiguousarray(np.asarray(b_, f32).reshape(NK, 128).T)
    bdp = (bv["bd"] + (b_vg + bv["bv_p"]) @ W["Wd"]).astype(f32)
    shared["bcol_d"] = np.ascontiguousarray(bdp.reshape(NK, 128).T)
    shared["bcol_fc"] = np.ascontiguousarray(bv["bfc"].reshape(NFF, 128).T)
    shared["bml_bc"] = np.ascontiguousarray(
        np.broadcast_to(bv["bml"].reshape(1, D), (128, D)))
    shared["w_fc"] = np.ascontiguousarray(
        W["Wfc"].reshape(NK, 128, NFF, 128).transpose(2, 0, 1, 3))
    shared["w_proj"] = np.ascontiguousarray(
        W["Wproj"].reshape(NFF, 128, NK, 128).transpose(2, 0, 1, 3))

    in_maps = []
    for b in range(B):
        m = dict(shared)
        m["xg"] = np.ascontiguousarray(g[:, b, :].T)
        m["xl"] = np.ascontiguousarray(l[:, b, :].T)
        m["xt"] = np.ascontiguousarray(t[:, b, :].T)
        in_maps.append(m)
    return in_maps, mean_b


_CACHE = {}


def get_built(repeat, mean_b):
    key = (repeat, tuple(sorted(mean_b.items())))
    if key not in _CACHE:
        _CACHE[key] = build(repeat=repeat, mean_b=mean_b)
    return _CACHE[key]


def run(inputs, repeat=1):
    in_maps, mean_b = _prep_host(inputs)
    cx = get_built(repeat, mean_b)
    results = bass2jax.run_bass_via_pjrt(cx.nc, in_maps, n_cores=B)
    return np.stack([results[b]["y"] for b in range(B)], axis=0)


def kernel(**inputs):
    return run(inputs, repeat=int(os.environ.get("BASS_NN_REPEAT", "1")))



# revision 2
# speedup vs baseline: 249.1503x; 249.1503x over previous
"""Trainium2 Bass kernel for nn_MEX_41386304864396 (dense transformer block).

Sharding: data-parallel over batch B=8 across 8 NeuronCores (one batch element
per core); weights replicated.  Host pre-transposes activations to [D, S] and
pre-folds constants so the device never transposes:
  * residual fold      W' = I + W                  (embed blocks LN(x + xW + b))
  * LN mean fold       extra matmul column: m = x @ (W'.sum(1)/D) + mean(b)
  * LN affine fold     gamma/beta folded into the downstream q/k/v weights
  * v-bias fold        bd' = bd + (bv_g + bv_p) @ Wd   (softmax rows sum to 1)
  * bridged attention  glb_ctx + plb_ctx = softmax_g @ (vg + softmax_p @ vp)
    (associativity removes the S x S x S 'enhanced' matmul)
Scores are computed transposed [key, query]; softmax uses exp without max
subtraction (scores bounded ~|3.4|) and normalization is deferred:
Z_p is a folded ones-column of the U_p matmul, Z_g a folded ones-row of U_g.

v2 changes vs v1:
  * bf16 operands everywhere (fp32 PSUM accumulation, fp32 LN statistics):
    halves weight DMA + SBUF, bf16 matmul runs 1 cycle/row at ANY output
    width (fp32r pays 4x below 256), and 16-bit DVE ops run 2x.
  * one persistent double-buffered weight ring (bufs=2) with each phase's
    weights DMA-prefetched one phase ahead; merged single-DMA loads.
  * partition broadcasts (LN rstd/mean, softmax 1/Z_g, final bias) moved off
    the PE onto the idle GpSimd/Pool engine (partition_broadcast ucode).
  * projection order kg,vg,qg,qp,kp,vp so each LayerNorm tail hides under
    matmul phases that don't depend on it.
"""
import os
import sys

sys.path.insert(0, '/opt/trn_rl_repo')

import numpy as np
import ml_dtypes

import concourse.bass as bass  # noqa: F401
import concourse.bass_isa as bass_isa
import concourse.tile as tile
from concourse import bacc, mybir
from concourse import bass2jax

F32 = mybir.dt.float32
BF = mybir.dt.bfloat16
AF = mybir.ActivationFunctionType
ALU = mybir.AluOpType

S, B, D, H, DH, FF = 512, 8, 1024, 16, 64, 4096
NK = D // 128
NT = S // 128
NFF = FF // 128
EPS = 1e-5
SCALE = 1.0 / 8.0
NPBF = ml_dtypes.bfloat16


def _declare(nc, timing=False):
    dram = {}
    kind = "Internal" if timing else "ExternalInput"

    def din(name, shape, dt=BF):
        dram[name] = nc.dram_tensor(name, list(shape), dt, kind=kind)

    for n in ("xg", "xl", "xt"):
        din(n, (D, S))
    for n in ("we_l1", "we_l2", "we_g", "w_qg", "w_kg", "w_qp", "w_kp",
              "w_vg", "w_vp", "w_d", "w_ml"):
        din(n, (D, D))
    din("w_fc", (NFF, 128, NK, 128))
    din("w_proj", (NK, 128, NFF, 128))
    din("cpack", (128, 104), F32)
    din("bml_row", (1, D), F32)
    dram["y"] = nc.dram_tensor("y", [S, D], F32, kind="ExternalOutput")
    return dram


def _body(nc, tc, dram, mean_b):
    def pool(name, bufs, side="left", space="SBUF"):
        return tc.alloc_tile_pool(name=name, bufs=bufs, side=side, space=space)

    # ---- global pools (bottom of left stack / PSUM) ----
    consts = pool("consts", 1)
    rows = pool("rows", 1)
    tmp = pool("tmp", 2)
    sq = pool("sqp", 4)
    small = pool("small", 3)

    psA = pool("psA", 6, space="PSUM")
    psUp = pool("psUp", 1, space="PSUM")
    psUg = pool("psUg", 1, space="PSUM")

    def mmtile():
        return psA.tile([128, 512], F32, tag="mm", name="mm")

    # ---- constants (tiny DMAs first so they never queue behind bulk) ----
    ones_f = consts.tile([128, 32], F32, tag="ones_f", name="ones_f")
    nc.vector.memset(ones_f[:], 1.0)
    ones_bf = consts.tile([128, 32], BF, tag="ones_bf", name="ones_bf")
    nc.vector.tensor_copy(ones_bf[:], ones_f[:])
    eps_t = consts.tile([1, 1], F32, tag="eps_t", name="eps_t")
    nc.vector.memset(eps_t[:], EPS)
    warm = consts.tile([128, 256], BF, tag="warm", name="warm")
    nc.vector.memset(warm[:], 0.0)

    def cload(name, shape, dt=F32):
        t = consts.tile(list(shape), dt, tag=name)
        nc.sync.dma_start(out=t[:], in_=dram[name].ap())
        return t

    # one packed DMA for all per-output bias columns (HWDGE dispatch is
    # 625ns/DMA; 11 tiny loads in front of xl would delay the PE start)
    cpack = cload("cpack", (128, 104))
    cnames = ["l1", "l2", "g", "qg", "kg", "qp", "kp", "d", "proj"]
    bcol_e = {e: cpack[:, 8 * i:8 * i + 8] for i, e in enumerate(cnames[:3])}
    bcols = {n: cpack[:, 8 * (3 + i):8 * (3 + i) + 8]
             for i, n in enumerate(cnames[3:])}
    bcol_fc = cpack[:, 72:104]
    bml_row = cload("bml_row", (1, D))

    # ---- weight ring: one phase prefetched ahead ----
    pw = pool("pw", 2)

    def wload(name, split=1):
        t = pw.tile([128, NK, D], BF, tag="wring", name=name)
        ap = dram[name].ap().rearrange("(k p) n -> p k n", p=128)
        step = NK // split
        for i in range(split):
            nc.sync.dma_start(out=t[:, i * step:(i + 1) * step, :],
                              in_=ap[:, i * step:(i + 1) * step, :])
        return t

    # ---- activations ----
    # projection output pools + pyln sit below px_gl so that px_gl (dead
    # after the embeds) and pyln (dead after vp) release in LIFO order
    pkp = pool("pkp", 1)
    pv = pool("pv", 1)
    pkg = pool("pkg", 1)
    pqg = pool("pqg", 1)
    pqp = pool("pqp", 1)
    # y-tags bufs=3: three embeds' y tiles live until the deferred norms run
    pyln = pool("pyln", 1)
    px_gl = pool("px_gl", 1)

    def xload(dname, p, tag):
        t = p.tile([128, NK, S], BF, tag=tag, name=tag)
        nc.sync.dma_start(
            out=t[:], in_=dram[dname].ap().rearrange("(k p) s -> p k s", p=128))
        return t

    xl = xload("xl", px_gl, "xl")
    wl1 = wload("we_l1", split=2)
    for w in range(60):
        wps = psA.tile([128, 256], F32, tag="mm", name="wmm")
        nc.tensor.matmul(wps[:], warm[:, 0:128], warm[:], start=True, stop=True)
    wl2 = wload("we_l2")
    xg = xload("xg", px_gl, "xg")


    # ---- embeds (plain LN; gamma/beta folded downstream on host) ----
    # LN row statistics come from GpSimd partition_all_reduce on the y / y^2
    # tiles (fp32 internal), not PE ones-matmuls: saves 24.6k PE rows and the
    # y bias makes the mean fold unnecessary.
    e_state = {}
    RADD = bass_isa.ReduceOp.add

    def embedA(e, x, wt):
        ys = []
        macc = rows.tile([1, 512], F32, tag="macc", name="macc", bufs=2)
        sacc = rows.tile([1, 512], F32, tag="sacc", name="sacc", bufs=2)
        for m in range(NK):
            ps = mmtile()
            for j in range(NK):
                k = (m + 1 + j) % NK
                nc.tensor.matmul(ps[:], wt[:, k, m * 128:(m + 1) * 128],
                                 x[:, k, :],
                                 start=(j == 0), stop=(j == NK - 1))
            # both PSUM evictions on Act so the DVE stays free for the
            # previous embed's LN chain (psA WAR would stall the PE otherwise)
            y = pyln.tile([128, 512], BF, tag=f"y{m}", name=f"y{m}", bufs=3)
            nc.scalar.activation(y[:], ps[:], AF.Identity,
                                 bias=bcol_e[e][:, m:m + 1], scale=1.0)
            s = sq.tile([128, 512], BF, tag="sq", name="sq")
            nc.scalar.activation(s[:], ps[:], AF.Square,
                                 bias=bcol_e[e][:, m:m + 1], scale=1.0)
            ary = tmp.tile([128, 512], BF, tag="ary", name="ary", bufs=2)
            nc.gpsimd.partition_all_reduce(ary[:], y[:], 128, RADD)
            ars = tmp.tile([128, 512], BF, tag="ars", name="ars", bufs=2)
            nc.gpsimd.partition_all_reduce(ars[:], s[:], 128, RADD)
            if m == 0:
                nc.vector.tensor_copy(macc[:], ary[0:1, :])
                nc.vector.tensor_copy(sacc[:], ars[0:1, :])
            else:
                nc.vector.tensor_add(macc[:], macc[:], ary[0:1, :])
                nc.vector.tensor_add(sacc[:], sacc[:], ars[0:1, :])
            ys.append(y)
        e_state[e] = (macc, sacc, ys)

    def embedB(e, epool, defer=False):
        macc, sacc, ys = e_state[e]
        m_sb = rows.tile([1, 512], F32, tag="m_sb", name="m_sb")
        nc.vector.tensor_scalar_mul(m_sb[:], macc[:], 1.0 / D)
        msq = rows.tile([1, 512], F32, tag="msq", name="msq")
        nc.vector.tensor_mul(msq[:], m_sb[:], m_sb[:])
        var = rows.tile([1, 512], F32, tag="var", name="var")
        nc.vector.scalar_tensor_tensor(out=var[:], in0=sacc[:], scalar=1.0 / D,
                                       in1=msq[:], op0=ALU.mult, op1=ALU.subtract)
        std = rows.tile([1, 512], F32, tag="std", name="std")
        nc.scalar.activation(std[:], var[:], AF.Sqrt, bias=eps_t[:], scale=1.0)
        rstd = rows.tile([1, 512], F32, tag="rstd", name="rstd")
        nc.vector.reciprocal(rstd[:], std[:])
        rstd_h = rows.tile([1, 512], BF, tag="rstd_h", name="rstd_h")
        nc.vector.tensor_copy(rstd_h[:], rstd[:])
        mr_h = rows.tile([1, 512], BF, tag="mr_h", name="mr_h")
        nc.vector.tensor_mul(mr_h[:], m_sb[:], rstd[:])
        rstd_bc = tmp.tile([128, 512], BF, tag="rstd_bc", name="rstd_bc")
        nc.gpsimd.partition_broadcast(rstd_bc[:], rstd_h[:])
        mr_bc = tmp.tile([128, 512], BF, tag="mr_bc", name="mr_bc")
        nc.gpsimd.partition_broadcast(mr_bc[:], mr_h[:])
        et = []

        def norm(m):
            t1 = sq.tile([128, 512], BF, tag="t1", name="t1", bufs=2)
            nc.vector.tensor_mul(t1[:], ys[m][:], rstd_bc[:])
            em = epool.tile([128, 512], BF, tag=f"e{m}", name=f"e{m}")
            nc.vector.tensor_sub(em[:], t1[:], mr_bc[:])
            et.append(em)

        e_state[e] = et
        if defer:
            return norm
        for m in range(NK):
            norm(m)

    pe_dup = pool("pe_dup", 1, side="right")
    pe_l2 = pool("pe_l2", 1, side="right")
    pe_g2 = pool("pe_g2", 1, side="right")
    px_t = pool("px_t", 1, side="right")

    # embedB(e) emitted one phase late so its DVE chain never delays the next
    # embedA's statistic row-adds (which pace the Pool reduces); the l2/g
    # norm tensor-ops are deferred and spread into the kp/vp loops so their
    # DVE lump never stalls projection PSUM evictions
    embedA("l1", xl, wl1)
    wg = wload("we_g")          # reuses wl1's ring slot
    embedA("l2", xl, wl2)
    embedB("l1", pe_dup)
    xt = xload("xt", px_t, "xt")
    embedA("g", xg, wg)
    norm_l2 = embedB("l2", pe_l2, defer=True)
    wkp = wload("w_kp")
    norm_g = embedB("g", pe_g2, defer=True)
    dupT = e_state["l1"]
    px_gl.release()

    # ---- projections (kg,vg first: l2 is ready before g2) ----
    def projB(wt, src, bcol, opool, tagp, inter=None, stagger=True):
        out = []
        for m in range(NK):
            ps = mmtile()
            for j in range(NK):
                k = (m + 1 + j) % NK if stagger else j
                nc.tensor.matmul(ps[:], wt[:, k, m * 128:(m + 1) * 128], src[k],
                                 start=(j == 0), stop=(j == NK - 1))
            o = opool.tile([128, 512], BF, tag=f"{tagp}{m}", name=f"{tagp}{m}")
            nc.vector.tensor_scalar_add(o[:], ps[:], bcol[:, m:m + 1])
            out.append(o)
            if inter:
                inter(m)
        return out

    def projA(wt, src, opool, tagp, width, inter=None):
        out = []
        for rt in range(NT):
            vt = opool.tile([128, H, width], BF, tag=f"{tagp}{rt}",
                            name=f"{tagp}{rt}")
            if width == DH + 2:
                for c in (DH, DH + 1):
                    nc.vector.tensor_copy(
                        vt[:, :, c:c + 1].rearrange("p h one -> p (h one)"),
                        ones_bf[:, 0:H])
            elif width == DH + 1:
                # zeros column: the vph stt then yields Zp*(1/Zp) + 0 = 1
                nc.vector.memset(
                    vt[:, :, DH:DH + 1].rearrange("p h one -> p (h one)"), 0.0)
            for half in range(2):
                ps = mmtile()
                for j in range(NK):
                    k = (2 * rt + half + 1 + j) % NK
                    nc.tensor.matmul(
                        ps[:], src[k][:, rt * 128:(rt + 1) * 128],
                        wt[:, k, half * 512:(half + 1) * 512],
                        start=(j == 0), stop=(j == NK - 1))
                nc.vector.tensor_copy(
                    vt[:, half * 8:(half + 1) * 8, 0:DH],
                    ps[:].rearrange("p (h d) -> p h d", h=8))
                if inter:
                    inter(2 * rt + half)
            out.append(vt)
        return out

    xtv = [xt[:, k, :] for k in range(NK)]

    def hsl(tiles, h):
        return tiles[h // 2][64 * (h % 2):64 * (h % 2) + 64, :]

    # precompute exp(scores_g) for all heads, interleaved into the vg/qp
    # projection loops: the Act engine is idle there but is the bottleneck
    # inside the attention loop (8 exp tiles/head vs 3us PE work).  Emitting
    # them as a solid block would flip the stall onto the PE (psA WAR waits
    # on Act eviction), so spread one head per projection step.
    PRE_H = 16
    peg_t = {}

    def emit_peg(h):
        qg_h, kg_h = hsl(qgT, h), hsl(kgT, h)
        for kt in range(NT):
            sg = mmtile()
            nc.tensor.matmul(sg[:], kg_h[:, kt * 128:(kt + 1) * 128], qg_h,
                             start=True, stop=True)
            e = peg.tile([128, 512], BF, tag=f"peg{h}_{kt}", name=f"peg{h}_{kt}")
            nc.scalar.activation(e[:], sg[:], AF.Exp, scale=SCALE)
            peg_t[(h, kt)] = e

    def peg_inter(sched):
        def inter(i):
            h = sched[i]
            if h is not None:
                emit_peg(h)
        return inter

    # kp, vp first: they depend only on xt, so the l2/g LayerNorm tails get
    # 27us of independent matmuls to hide under before kg needs them
    kpT = projB(wkp, xtv, bcols["kp"], pkp, "kp", inter=norm_l2)
    wvp = wload("w_vp")
    vp = projA(wvp, xtv, pv, "vpn", DH + 2, inter=norm_g)
    l2T, g2T = e_state["l2"], e_state["g"]
    pyln.release()
    wkg = wload("w_kg")
    px_t.release()
    kgT = projB(wkg, l2T, bcols["kg"], pkg, "kg")
    wqg = wload("w_qg")
    qgT = projB(wqg, g2T, bcols["qg"], pqg, "qg")
    wvg = wload("w_vg")
    pe_g2.release()
    peg = pool("peg", 1)
    vg = projA(wvg, l2T, pv, "vg", DH + 1,
               inter=peg_inter([0, 1, 2, 3, 4, 5, 6, 7]))
    wqp = wload("w_qp")
    pe_l2.release()
    qpT = projB(wqp, dupT, bcols["qp"], pqp, "qp",
                inter=peg_inter([8, 9, 10, 11, 12, 13, 14, 15]))
    wd = wload("w_d")
    pe_dup.release()
    wml = wload("w_ml")

    # ---- attention ----
    pctx = pool("pctx", 1, side="right")
    pexp = pool("pexp", 2, side="right")

    ctxT = [pctx.tile([128, 512], BF, tag=f"ctx{j}", name=f"ctx{j}")
            for j in range(NK)]
    stage1_out = {}

    stage_ep = {}

    def emit_ep(h):
        qp_h, kp_h = hsl(qpT, h), hsl(kpT, h)
        ep = []
        for kt in range(NT):
            sp = mmtile()
            nc.tensor.matmul(sp[:], kp_h[:, kt * 128:(kt + 1) * 128], qp_h,
                             start=True, stop=True)
            e = pexp.tile([128, 512], BF, tag=f"ep{kt}", name=f"ep{kt}", bufs=3)
            nc.scalar.activation(e[:], sp[:], AF.Exp, scale=SCALE)
            ep.append(e)
        stage_ep[h] = ep

    def emit_upvph(h):
        ep = stage_ep.pop(h)
        # all 16 accumulation groups first, THEN the vph chains: the DVE
        # work per kt (~1us) would otherwise stall the next kt's short
        # (66-row) bf16 matmul group via the shared-PSUM-tile dep
        up = psUp.tile([128, NT, DH + 2], F32, tag="up", name="up")
        for kt in range(NT):
            for tt in range(NT):
                nc.tensor.matmul(up[:, kt, :], ep[tt][:, kt * 128:(kt + 1) * 128],
                                 vp[tt][:, h, 0:DH + 2], start=(tt == 0),
                                 stop=(tt == NT - 1))
        rp4 = small.tile([128, NT], F32, tag="rp", name="rp")
        nc.vector.reciprocal(
            rp4[:], up[:, :, DH:DH + 1].rearrange("p k one -> p (k one)"))
        vph = []
        for kt in range(NT):
            vt = pexp.tile([128, DH + 1], BF, tag=f"vph{kt}", name=f"vph{kt}")
            nc.vector.scalar_tensor_tensor(
                out=vt[:], in0=up[:, kt, 0:DH + 1], scalar=rp4[:, kt:kt + 1],
                in1=vg[kt][:, h, 0:DH + 1], op0=ALU.mult, op1=ALU.add)
            vph.append(vt)
        stage1_out[h] = vph

    def attn_stage2(h):
        eg = [peg_t.pop((h, kt)) for kt in range(NT)]
        vph = stage1_out.pop(h)
        ug = psUg.tile([DH + 1, 512], F32, tag="ug", name="ug")
        for kt in range(NT):
            nc.tensor.matmul(ug[:], vph[kt][:], eg[kt][:],
                             start=(kt == 0), stop=(kt == NT - 1))
        rg_h = rows.tile([1, 512], BF, tag="rg_h", name="rg_h")
        nc.vector.reciprocal(rg_h[:], ug[DH:DH + 1, :])
        rbs = tmp.tile([64, 512], BF, tag="rbs", name="rbs")
        nc.gpsimd.partition_broadcast(rbs[:], rg_h[:])
        off = 64 * (h % 2)
        nc.vector.tensor_mul(ctxT[h // 2][off:off + 64, :], ug[0:DH, :], rbs[:])

    emit_ep(0)
    emit_ep(1)
    emit_ep(2)
    emit_upvph(0)
    for h in range(H):
        if h + 3 < H:
            emit_ep(h + 3)
        if h + 1 < H:
            emit_upvph(h + 1)
        attn_stage2(h)
    pexp.release()
    peg.release()
    pqp.release()
    pqg.release()
    pkg.release()
    pv.release()
    pkp.release()

    # ---- w_d + MLP (first weight blocks DMA'd under the w_d compute) ----
    outp = pool("outp", 2)
    bml_bc = outp.tile([128, D], F32, tag="bml_bc", name="bml_bc", bufs=1)
    nc.gpsimd.partition_broadcast(bml_bc[:], bml_row[:])
    po1 = pool("po1", 1)
    ph1 = pool("ph1", 1)
    po2 = pool("po2", 1)
    pwfc = pool("pwfc", 2)
    pwpj = pool("pwpj", 2)

    def fc_dma(blk):
        wt = pwfc.tile([128, 4, NK, 128], BF, tag="wfc", name="wfc")
        nc.sync.dma_start(
            out=wt[:],
            in_=dram["w_fc"].ap()[4 * blk:4 * blk + 4].rearrange(
                "f p k n -> p f k n"))
        return wt

    def pj_dma(m):
        wt = pwpj.tile([128, NFF, 128], BF, tag="wpj", name="wpj")
        nc.sync.dma_start(out=wt[:], in_=dram["w_proj"].ap()[m])
        return wt

    fc_wts = [fc_dma(0), fc_dma(1)]
    pj_wts = [pj_dma(0)]

    # k=j order: ctx chunks complete in head order, so visit them in order
    # and the last heads' LayerNorm-style tail hides under the first chunks
    out1T = projB(wd, ctxT, bcols["d"], po1, "o1", stagger=False)
    pctx.release()

    h1 = []
    for blk in range(NFF // 4):
        if blk + 2 < NFF // 4:
            fc_wts.append(fc_dma(blk + 2))
        wt = fc_wts[blk]
        for i in range(4):
            ff = 4 * blk + i
            ps = mmtile()
            for k in range(NK):
                nc.tensor.matmul(ps[:], wt[:, i, k, :], out1T[k],
                                 start=(k == 0), stop=(k == NK - 1))
            g = ph1.tile([128, 512], BF, tag=f"h1_{ff}", name=f"h1_{ff}")
            nc.scalar.activation(g[:], ps[:], AF.Gelu,
                                 bias=bcol_fc[:, ff:ff + 1], scale=1.0)
            h1.append(g)

    out2T = []
    for m in range(NK):
        if m + 1 < NK:
            pj_wts.append(pj_dma(m + 1))
        wt = pj_wts[m]
        ps = mmtile()
        for k in range(NFF):
            nc.tensor.matmul(ps[:], wt[:, k, :], h1[k],
                             start=(k == 0), stop=(k == NFF - 1))
        o = po2.tile([128, 512], BF, tag=f"o2m{m}", name=f"o2m{m}")
        nc.vector.tensor_scalar_add(o[:], ps[:], bcols["proj"][:, m:m + 1])
        out2T.append(o)

    # ---- y = out2 @ Wml + bml (natural layout) ----
    for rt in range(NT):
        yt = outp.tile([128, D], F32, tag="yout", name="yout")
        for half in range(2):
            ps = mmtile()
            for j in range(NK):
                k = (2 * rt + half + 1 + j) % NK
                nc.tensor.matmul(ps[:], out2T[k][:, rt * 128:(rt + 1) * 128],
                                 wml[:, k, half * 512:(half + 1) * 512],
                                 start=(j == 0), stop=(j == NK - 1))
            nc.vector.tensor_add(yt[:, half * 512:(half + 1) * 512], ps[:],
                                 bml_bc[:, half * 512:(half + 1) * 512])
            nc.sync.dma_start(
                out=dram["y"].ap()[rt * 128:(rt + 1) * 128,
                                   half * 512:(half + 1) * 512],
                in_=yt[:, half * 512:(half + 1) * 512])
    pwpj.release()
    pwfc.release()
    po2.release()
    ph1.release()
    po1.release()
    outp.release()
    pw.release()

    for p in (small, sq, tmp, rows, consts, psUg, psUp, psA):
        p.release()


def build(repeat=1, mean_b=None, timing=False):
    mean_b = mean_b or {"l1": 0.0, "l2": 0.0, "g": 0.0}
    nc = bacc.Bacc(None, target_bir_lowering=False, debug=False)
    dram = _declare(nc, timing=timing)
    with tile.TileContext(nc) as tc:
        with nc.allow_low_precision(reason="bf16 operands, fp32 accumulation"):
            if repeat > 1:
                with tc.For_i(0, repeat, 1):
                    _body(nc, tc, dram, mean_b)
            else:
                _body(nc, tc, dram, mean_b)
    nc.compile()

    class CX:
        pass

    cx = CX()
    cx.nc = nc
    cx.dram = dram
    return cx


# ---------------------------------------------------------------------------
# host side
# ---------------------------------------------------------------------------

def _prep_host(inputs):
    f32 = np.float32
    g = np.asarray(inputs["global_feat"], f32)
    l = np.asarray(inputs["local_feat"], f32)
    t = np.asarray(inputs["text_feat"], f32)
    W = {k: np.asarray(inputs[k], f32) for k in
         ("Wg_emb", "Wl1", "Wl2", "Wq_g", "Wk_g", "Wv_g", "Wq_p", "Wk_p",
          "Wv_p", "Wd", "Wml", "Wfc", "Wproj")}
    bv = {k: np.asarray(inputs[k], f32) for k in
          ("bg_emb", "bl1", "bl2", "bq_g", "bk_g", "bv_g", "bq_p", "bk_p",
           "bv_p", "bd", "bml", "bproj", "bfc",
           "betag_emb", "betal1", "betal2", "gg_emb", "gl1", "gl2")}

    def bf(a):
        return np.ascontiguousarray(np.asarray(a, f32).astype(NPBF))

    I = np.eye(D, dtype=f32)
    shared = {}
    mean_b = {}
    cpack_cols = []
    for e, (wn, bn) in {"l1": ("Wl1", "bl1"), "l2": ("Wl2", "bl2"),
                        "g": ("Wg_emb", "bg_emb")}.items():
        Wp = (I + W[wn]).astype(f32)
        shared[f"we_{e}"] = bf(Wp)
        mean_b[e] = float(bv[bn].mean())
        cpack_cols.append(bv[bn].reshape(NK, 128).T)

    # fold LN gamma/beta of the producing embed into each consumer projection
    def foldp(Wname, bname, gamma, beta):
        Wf = (gamma[:, None] * W[Wname]).astype(f32)
        bf_ = (np.asarray(bv[bname]) + beta @ W[Wname]).astype(f32)
        return Wf, bf_

    w_qg, b_qg = foldp("Wq_g", "bq_g", bv["gg_emb"], bv["betag_emb"])
    w_kg, b_kg = foldp("Wk_g", "bk_g", bv["gl2"], bv["betal2"])
    w_vg, b_vg = foldp("Wv_g", "bv_g", bv["gl2"], bv["betal2"])
    w_qp, b_qp = foldp("Wq_p", "bq_p", bv["gl1"], bv["betal1"])
    shared["w_qg"] = bf(w_qg)
    shared["w_kg"] = bf(w_kg)
    shared["w_qp"] = bf(w_qp)
    shared["w_kp"] = bf(W["Wk_p"])
    shared["w_vg"] = bf(w_vg)
    shared["w_vp"] = bf(W["Wv_p"])
    shared["w_d"] = bf(W["Wd"])
    shared["w_ml"] = bf(W["Wml"])
    bdp = (bv["bd"] + (b_vg + bv["bv_p"]) @ W["Wd"]).astype(f32)
    # cpack column order: bcol l1,l2,g | qg,kg,qp,kp,d,proj | fc
    for b_ in (b_qg, b_kg, b_qp, bv["bk_p"], bdp, bv["bproj"]):
        cpack_cols.append(np.asarray(b_, f32).reshape(NK, 128).T)
    cpack_cols.append(bv["bfc"].reshape(NFF, 128).T)
    shared["cpack"] = np.ascontiguousarray(
        np.concatenate(cpack_cols, axis=1).astype(f32))
    shared["bml_row"] = np.ascontiguousarray(bv["bml"].reshape(1, D))
    # [f, p, k, n] = Wfc[k*128+p, f*128+n]
    shared["w_fc"] = bf(
        W["Wfc"].reshape(NK, 128, NFF, 128).transpose(2, 1, 0, 3))
    # [m, p, k, n] = Wproj[k*128+p, m*128+n]
    shared["w_proj"] = bf(
        W["Wproj"].reshape(NFF, 128, NK, 128).transpose(2, 1, 0, 3))

    in_maps = []
    for b in range(B):
        m = dict(shared)
        m["xg"] = bf(g[:, b, :].T)
        m["xl"] = bf(l[:, b, :].T)
        m["xt"] = bf(t[:, b, :].T)
        in_maps.append(m)
    return in_maps, mean_b


_CACHE = {}


def get_built(repeat, mean_b):
    key = (repeat, tuple(sorted(mean_b.items())))
    if key not in _CACHE:
        _CACHE[key] = build(repeat=repeat, mean_b=mean_b)
    return _CACHE[key]


def run(inputs, repeat=1):
    in_maps, mean_b = _prep_host(inputs)
    cx = get_built(repeat, mean_b)
    results = bass2jax.run_bass_via_pjrt(cx.nc, in_maps, n_cores=B)
    return np.stack([results[b]["y"] for b in range(B)], axis=0)


def kernel(**inputs):
    return run(inputs, repeat=int(os.environ.get("BASS_NN_REPEAT", "1")))
